# revision 53
# baseline (speedup 1.0000x reference)
"""FFTBlock (attention + conv-FFN transformer block) on 8 Trainium2 NeuronCores.

Data-parallel over batch: 16 batch items -> 2 per core. Each core runs the
full block (MHA + LN + conv1d-FFN + LN) on its 2 batch items.

v2 changes over baseline:
  - All weights pre-transposed on host -> every weight DMA is a linear slab
    (the rearrange-DMAs were 384B-packet gathers that stalled startup).
  - Attention restructured as a kc-pipelined (flash-style) loop: per key tile
    scores -> exp -> mask -> A@V accumulate into persistent PSUM, with lag-2
    software pipelining so the PE never waits on the scalar/vector chain.
  - Mask loaded once per batch item (not per head), prefetched at phase start.
  - Residual (xn) tiles prefetched at phase start.
  - Out-projection/LN units decoupled from the x1T transposes (interleaved so
    the PE transpose never waits on the LN vector chain).
  - w2 (conv2 weights, 10.6MB) DMA split per-tap and interleaved with
    conv1(b1) so it no longer stalls the phase-3 boundary.
"""

import sys

sys.path.insert(0, "/opt/trn_rl_repo")

import math
from contextlib import ExitStack

import ml_dtypes
import numpy as np

import concourse.bass as bass
import concourse.mybir as mybir
import concourse.tile as tile
from concourse import bacc
from concourse.bass_utils import run_bass_kernel_spmd
from concourse.masks import make_identity

BF16 = mybir.dt.bfloat16
F32 = mybir.dt.float32
F8 = mybir.dt.float8e4
DR = mybir.MatmulPerfMode.DoubleRow
AF = mybir.ActivationFunctionType
ALU = mybir.AluOpType

B, S, D, H, DH, F, K = 16, 1024, 384, 2, 192, 1536, 9
NCORES = 8
NB = B // NCORES  # batch items per core
EPS = 1e-5
ISCALE = 1.0 / math.sqrt(D)  # NOTE: reference scales by sqrt(d_model)
SP = S + 8  # padded sequence length (4 left, 4 right)
DC = D // 128  # 3 d-chunks
FT = F // 128  # 12 filter tiles
ST = S // 128  # 8 seq tiles of 128
SQ = S // 512  # 2 seq chunks of 512

_CACHE = {}


def _bcast(ap, p=128):
    return bass.AP(tensor=ap.tensor, offset=ap.offset, ap=[[0, p]] + list(ap.ap))


def _emit(nc):
    # ---- DRAM I/O (all host-pretransposed: partition dim first) ----
    xT_d = nc.dram_tensor("xT", [NB, 128, DC, S], F8, kind="ExternalInput")
    xn_d = nc.dram_tensor("xn", [NB, 128, ST, D], F32, kind="ExternalInput")
    mT_d = nc.dram_tensor("mT", [NB, 128, ST, S], F8, kind="ExternalInput")
    wq_d = nc.dram_tensor("wq", [128, H, DC, DH], F8, kind="ExternalInput")
    wk_d = nc.dram_tensor("wk", [128, H, DC, DH], F8, kind="ExternalInput")
    wv_d = nc.dram_tensor("wv", [128, H, DC, DH], F8, kind="ExternalInput")
    wo_d = nc.dram_tensor("wo", [128, 4, D], F8, kind="ExternalInput")
    wc1_d = nc.dram_tensor("wc1", [FT, 128, K, DC, 128], BF16, kind="ExternalInput")
    wc2_d = nc.dram_tensor("wc2", [128, K, FT, D], BF16, kind="ExternalInput")
    bqk_d = nc.dram_tensor("bqk", [128, 2, H, 2], F32, kind="ExternalInput")
    bv_d = nc.dram_tensor("bv", [H, DH], F32, kind="ExternalInput")
    bc1_d = nc.dram_tensor("bc1t", [128, FT], F32, kind="ExternalInput")
    lnc_d = nc.dram_tensor("lnc", [5, D], F32, kind="ExternalInput")
    y_d = nc.dram_tensor("y", [NB, ST, 128, D], F32, kind="ExternalOutput")

    with tile.TileContext(nc) as tc:
        _body(nc, tc, locals())
    nc.finalize()
    return nc


def _body(nc, tc, d):
    xT_d, xn_d, mT_d = d["xT_d"], d["xn_d"], d["mT_d"]
    wq_d, wk_d, wv_d, wo_d = d["wq_d"], d["wk_d"], d["wv_d"], d["wo_d"]
    wc1_d, wc2_d = d["wc1_d"], d["wc2_d"]
    bqk_d, bv_d, bc1_d = d["bqk_d"], d["bv_d"], d["bc1_d"]
    lnc_d, y_d = d["lnc_d"], d["y_d"]

    with ExitStack() as ctx:
        const = ctx.enter_context(tc.tile_pool(name="const", bufs=1))
        persist = ctx.enter_context(tc.tile_pool(name="persist", bufs=1))

        # ---- critical-path weights first (all linear slabs now) ----
        wq_sb = const.tile([128, H, DC, DH], F8, tag="wq")
        nc.sync.dma_start(wq_sb[:], wq_d[:])
        bqk_sb = const.tile([128, 2, H, 2], F32, tag="bqk")
        nc.sync.dma_start(bqk_sb[:], bqk_d[:])

        # ---- phased execution ----
        # P1: attention(b0) + qkv(b1) filler   P2: attention(b1) || conv1(b0)
        # P3: conv1(b1) || [w2 load chunks] then conv2(b0)   P4: conv2(b1)
        with ExitStack() as octx:
            qkvp1 = octx.enter_context(tc.tile_pool(name="qkvp1", bufs=1))
            xTp = octx.enter_context(tc.tile_pool(name="xTp", bufs=1))
            XT = {}
            for b in range(NB):
                XT[b] = xTp.tile([128, DC, S], F8, name=f"xT{b}", tag=f"xT{b}")
                nc.sync.dma_start(XT[b][:], xT_d[b])
            wk_sb = const.tile([128, H, DC, DH], F8, tag="wk")
            nc.sync.dma_start(wk_sb[:], wk_d[:])
            wv_sb = const.tile([128, H, DC, DH], F8, tag="wv")
            nc.sync.dma_start(wv_sb[:], wv_d[:])
            bv_sb = const.tile([128, H, DH], F32, tag="bv")
            nc.sync.dma_start(bv_sb[:], _bcast(bv_d[:]))

            # remaining constants (off the critical path)
            wo_sb = const.tile([128, 4, D], F8, tag="wo")
            nc.sync.dma_start(wo_sb[:], wo_d[:])
            ident = const.tile([128, 128], F32, tag="ident")
            make_identity(nc, ident[:])
            bc1_sb = const.tile([128, FT], F32, tag="bc1")
            nc.sync.dma_start(bc1_sb[:], bc1_d[:])
            lnc_sb = const.tile([128, 5, D], F32, tag="lnc")
            nc.sync.dma_start(lnc_sb[:], _bcast(lnc_d[:]))
            g1_sb, be1_sb = lnc_sb[:, 0, :], lnc_sb[:, 1, :]
            g2_sb, be2_sb = lnc_sb[:, 2, :], lnc_sb[:, 3, :]
            bc2_sb = lnc_sb[:, 4, :]
            eps_sb = const.tile([128, 1], F32, tag="eps")
            nc.vector.memset(eps_sb[:], EPS)

            x1T = persist.tile([128, NB, DC, SP], BF16, tag="x1T")
            x1n = persist.tile([128, NB, ST, D], F32, tag="x1n")
            for b in range(NB):
                nc.gpsimd.memset(x1T[:, b, :, 0:4], 0.0)
                nc.gpsimd.memset(x1T[:, b, :, 4 + S : SP], 0.0)

            QT, KT, VV, ON = {}, {}, {}, {}

            def weave(a, b):
                # proportional merge of two unit lists; emits every closure
                ia = ib = 0
                while ia < len(a) or ib < len(b):
                    if ib >= len(b) or (ia < len(a) and ia * (len(b) + 1) <= ib * (len(a) + 1)):
                        a[ia](); ia += 1
                    else:
                        b[ib](); ib += 1

            def qkv_units(b, qkvp, psA, smal):
                units = []
                for h in range(H):
                    qk_us, v_us = [], []
                    # fp8 Q,K hold 8*(Q+bq); rows 64:128 of the second d-chunk
                    # are zeroed so DoubleRow's full-128 contraction adds 0.
                    qt = qkvp.tile([128, 2, S], F8, name=f"qt{b}{h}", tag=f"qt{b}{h}")
                    kt = qkvp.tile([128, 2, S], F8, name=f"kt{b}{h}", tag=f"kt{b}{h}")
                    # fp8 V holds 16*(V+bv) + a ones column; free dim padded to
                    # 208 so DoubleRow kc-pair strides are 16B-aligned.
                    vv = qkvp.tile([128, ST, 208], F8, name=f"vv{b}{h}", tag=f"vv{b}{h}")
                    QT[b, h], KT[b, h], VV[b, h] = qt, kt, vv
                    nc.gpsimd.memset(qt[64:128, 1, :], 0.0)
                    nc.gpsimd.memset(kt[64:128, 1, :], 0.0)
                    for wsb, bi, dst in ((wq_sb, 0, qt), (wk_sb, 1, kt)):
                        for mc, (m0, msz) in enumerate(((0, 128), (128, 64))):
                            for qc in range(SQ):
                                def u(b=b, h=h, wsb=wsb, bi=bi, dst=dst, m0=m0, msz=msz, mc=mc, qc=qc):
                                    ps = psA.tile([128, 512], F32, name="psqk", tag="p512")
                                    qs = slice(qc * 512, qc * 512 + 512)
                                    nc.tensor.matmul(
                                        ps[:msz, :],
                                        lhsT=wsb[:, h, 0:2, m0 : m0 + msz],
                                        rhs=XT[b][:, 0:2, qs],
                                        start=True,
                                        stop=False,
                                        perf_mode=DR,
                                    )
                                    nc.tensor.matmul(
                                        ps[:msz, :],
                                        lhsT=wsb[:, h, 2, m0 : m0 + msz],
                                        rhs=XT[b][:, 2, qs],
                                        start=False,
                                        stop=True,
                                    )
                                    nc.scalar.activation(
                                        out=dst[:msz, mc, qc * 512 : qc * 512 + 512],
                                        in_=ps[:msz, :],
                                        func=AF.Identity,
                                        bias=bqk_sb[:msz, bi, h, mc : mc + 1],
                                        scale=1.0 / 64.0,
                                    )
                                qk_us.append(u)
                    for st in range(ST):
                        def u(b=b, h=h, vv=vv, st=st):
                            ps = psA.tile([128, 512], F32, name="psv", tag="p512")
                            ss = slice(st * 128, st * 128 + 128)
                            nc.tensor.matmul(
                                ps[:, :DH],
                                lhsT=XT[b][:, 0:2, ss],
                                rhs=wv_sb[:, h, 0:2, :],
                                start=True,
                                stop=False,
                                perf_mode=DR,
                            )
                            nc.tensor.matmul(
                                ps[:, :DH],
                                lhsT=XT[b][:, 2, ss],
                                rhs=wv_sb[:, h, 2, :],
                                start=False,
                                stop=True,
                            )
                            # psum = 1024*xWv -> vv = fp8(ps/64 + 16bv)
                            tmp = smal.tile([128, DH], F32, name="vtmp", tag="vtmp")
                            nc.vector.tensor_scalar(
                                out=tmp[:], in0=ps[:, :DH], scalar1=1.0 / 64.0,
                                scalar2=None, op0=ALU.mult,
                            )
                            nc.vector.tensor_add(
                                out=vv[:, st, 0:DH], in0=tmp[:], in1=bv_sb[:, h, :]
                            )
                            nc.gpsimd.memset(vv[:, st, DH : DH + 1], 1.0)
                        v_us.append(u)
                    # alternate QK (scalar-consumer) and V (vector-consumer)
                    # units so psum-buffer reuse rarely waits on either engine
                    for i in range(max(len(qk_us), len(v_us))):
                        if i < len(qk_us):
                            units.append(qk_us[i])
                        if i < len(v_us):
                            units.append(v_us[i])
                return units

            def attn_units(b, expp, mskp, smal, lnp, xnp, psB, psC, attn):
                """kc-pipelined attention for item b.

                Per (h, kc, qc): scores matmul -> exp -> mask-mul, with the
                A@V accumulation for step i-2 emitted after step i's scores so
                the PE never waits on the scalar/vector chain (lag-2 pipeline).
                """
                units = []
                # mask + residual prefetch (one unit; DMAs overlap early scores)
                mask_sb = mskp.tile([128, ST, S], F8, name=f"msk{b}", tag=f"msk{b}")
                xn_sb = xnp.tile([128, ST, D], F32, name=f"xn{b}", tag=f"xn{b}")

                def u_prefetch(b=b, mask_sb=mask_sb, xn_sb=xn_sb):
                    nc.sync.dma_start(mask_sb[:], mT_d[b])
                    nc.sync.dma_start(xn_sb[:], xn_d[b])
                units.append(u_prefetch)

                # AV accumulators: 4 persistent PSUM banks (per qc: full + tail)
                acc = {}

                def sc_unit(b, h, kc, qc, expT):
                    qt, kt = QT[b, h], KT[b, h]
                    qs = slice(qc * 512, qc * 512 + 512)
                    ps = psB.tile([128, 512], F32, name="pssc", tag="sc")
                    # fp8 DoubleRow: both 96-deep halves (zero-padded to 128)
                    # in one pass; psum = 64 * scores.
                    nc.tensor.matmul(
                        ps[:, :],
                        lhsT=kt[:, :, kc * 128 : kc * 128 + 128],
                        rhs=qt[:, :, qs],
                        start=True,
                        stop=True,
                        perf_mode=DR,
                    )
                    nc.scalar.activation(
                        out=expT[:, kc, qs], in_=ps[:], func=AF.Exp, scale=ISCALE / 64.0,
                    )
                    nc.vector.tensor_mul(
                        out=expT[:, kc, qs], in0=expT[:, kc, qs], in1=mask_sb[:, kc, qs]
                    )

                def av_unit(b, h, kp, qc, expT):
                    # fp8 DoubleRow over a pair of key tiles (2kp, 2kp+1)
                    vv = VV[b, h]
                    qs = slice(qc * 512, qc * 512 + 512)
                    ps0, ps1 = acc[qc]
                    nc.tensor.matmul(
                        ps0[:],
                        lhsT=vv[:, 2 * kp : 2 * kp + 2, 0:128],
                        rhs=expT[:, 2 * kp : 2 * kp + 2, qs],
                        start=(kp == 0),
                        stop=(kp == ST // 2 - 1),
                        perf_mode=DR,
                    )
                    nc.tensor.matmul(
                        ps1[:65, :],
                        lhsT=vv[:, 2 * kp : 2 * kp + 2, 128 : DH + 1],
                        rhs=expT[:, 2 * kp : 2 * kp + 2, qs],
                        start=(kp == 0),
                        stop=(kp == ST // 2 - 1),
                        perf_mode=DR,
                    )

                for h in range(H):
                    expT = expp.tile([128, ST, S], F8, name=f"expT{h}", tag="expT")
                    onrm = attn.tile([128, 2, S], F8, name=f"on{b}{h}", tag=f"on{b}{h}")
                    ON[b, h] = onrm
                    nc.gpsimd.memset(onrm[64:128, 1, :], 0.0)

                    def u_alloc(h=h, b=b):
                        for qc in range(SQ):
                            ps0 = psC.tile([128, 512], F32, name=f"av0q{qc}", tag=f"av0q{qc}")
                            ps1 = psC.tile([128, 512], F32, name=f"av1q{qc}", tag=f"av1q{qc}")
                            acc[qc] = (ps0, ps1)

                    steps = [(kc, qc) for kc in range(ST) for qc in range(SQ)]
                    # av step (kp, qc) needs mask(2kp+1, qc) done, i.e. sc step
                    # index 2*(2kp+1)+qc; emit one sc later for pipeline slack.
                    av_after = {}
                    av_tail = []
                    for kp in range(ST // 2):
                        for qc in range(SQ):
                            gate = 2 * (2 * kp + 1) + qc + 1
                            if gate < len(steps):
                                av_after.setdefault(gate, []).append((kp, qc))
                            else:
                                av_tail.append((kp, qc))

                    def mk(i, h=h, expT=expT):
                        def u(i=i, h=h, expT=expT):
                            if i == 0:
                                u_alloc(h=h)
                            sc_unit(b, h, *steps[i], expT)
                            for kp, qc in av_after.get(i, ()):
                                av_unit(b, h, kp, qc, expT)
                        return u

                    for i in range(len(steps)):
                        units.append(mk(i))

                    def u_tail(b=b, h=h, expT=expT, onrm=onrm, av_tail=av_tail):
                        for kp, qc in av_tail:
                            av_unit(b, h, kp, qc, expT)
                        for qc in range(SQ):
                            qs = slice(qc * 512, qc * 512 + 512)
                            ps0, ps1 = acc[qc]
                            rc = smal.tile([1, 512], F32, tag="rc")
                            nc.scalar.copy(out=rc[:], in_=ps1[64:65, :])
                            rb = smal.tile([128, 512], F32, tag="rb")
                            nc.gpsimd.partition_broadcast(rb[:], rc[:])
                            nc.vector.reciprocal(rb[:], rb[:])
                            nc.vector.tensor_mul(out=onrm[:, 0, qs], in0=ps0[:], in1=rb[:])
                            nc.vector.tensor_mul(
                                out=onrm[:64, 1, qs], in0=ps1[:64, :], in1=rb[:64, :]
                            )
                    units.append(u_tail)

                # out-projection + LN (no transposes inside)
                def op_unit(st, b=b, xn_sb=xn_sb):
                    ps = psB.tile([128, 512], F32, name="at", tag="sc")
                    # fp8 DoubleRow per head; psum = 1024*attn, residual is
                    # host-prescaled by 1024 (layernorm is scale-invariant).
                    for h in range(H):
                        nc.tensor.matmul(
                            ps[:, :D],
                            lhsT=ON[b, h][:, :, st * 128 : st * 128 + 128],
                            rhs=wo_sb[:, 2 * h : 2 * h + 2, :],
                            start=(h == 0),
                            stop=(h == H - 1),
                            perf_mode=DR,
                        )
                    t = lnp.tile([128, D], F32, tag="t")
                    nc.vector.tensor_add(out=t[:], in0=ps[:, :D], in1=xn_sb[:, st, :])
                    stats = lnp.tile([128, 6], F32, tag="st")
                    nc.vector.bn_stats(out=stats[:], in_=t[:])
                    mv = lnp.tile([128, 2], F32, tag="mv")
                    nc.vector.bn_aggr(out=mv[:], in_=stats[:])
                    sd = lnp.tile([128, 1], F32, tag="sd")
                    nc.scalar.activation(
                        out=sd[:], in_=mv[:, 1:2], func=AF.Sqrt, bias=eps_sb[:],
                    )
                    nc.vector.reciprocal(sd[:], sd[:])
                    xv = x1n[:, b, st, :]
                    nc.vector.tensor_scalar(
                        out=xv, in0=t[:], scalar1=mv[:, 0:1], scalar2=sd[:],
                        op0=ALU.subtract, op1=ALU.mult,
                    )
                    nc.vector.tensor_mul(out=xv, in0=xv, in1=g1_sb)
                    nc.vector.tensor_add(out=xv, in0=xv, in1=be1_sb)

                def tr_unit(st, b=b):
                    for dc in range(DC):
                        tp = psB.tile([128, 512], F32, name="tp", tag="sc")
                        nc.tensor.transpose(
                            tp[:, :128], x1n[:, b, st, dc * 128 : dc * 128 + 128], ident[:]
                        )
                        nc.scalar.copy(
                            out=x1T[:, b, dc, 4 + st * 128 : 4 + st * 128 + 128],
                            in_=tp[:, :128],
                        )

                # transpose(st) runs three units after its LN chain, so the
                # PE never waits on the vector LN pipeline.
                LAG = 3
                sched = [("op", st) for st in range(LAG)]
                for st in range(LAG, ST):
                    sched += [("op", st), ("tr", st - LAG)]
                sched += [("tr", st) for st in range(ST - LAG, ST)]
                for kind, st in sched:
                    units.append(
                        (lambda st=st: op_unit(st)) if kind == "op"
                        else (lambda st=st: tr_unit(st))
                    )
                return units

            def conv1_units(b, w1p, psF, hT, extra_dma=None):
                units = []
                for ft in range(FT):
                    def udma(ft=ft):
                        w1 = w1p.tile([128, K, DC, 128], BF16, name="w1", tag="w1")
                        conv1_units._w1 = w1
                        nc.sync.dma_start(w1[:], wc1_d[ft])
                        if extra_dma is not None and ft < len(extra_dma):
                            extra_dma[ft]()
                    units.append(udma)
                    for qc in range(SQ):
                        def u(b=b, ft=ft, qc=qc):
                            w1 = conv1_units._w1
                            ps = psF.tile([128, 512], F32, name="c1", tag="c1")
                            idx = 0
                            for k9 in range(K):
                                for dc in range(DC):
                                    nc.tensor.matmul(
                                        ps[:],
                                        lhsT=w1[:, k9, dc, :],
                                        rhs=x1T[:, b, dc, qc * 512 + k9 : qc * 512 + k9 + 512],
                                        start=(idx == 0),
                                        stop=(idx == K * DC - 1),
                                    )
                                    idx += 1
                            nc.scalar.activation(
                                out=hT[:, ft, 4 + qc * 512 : 4 + qc * 512 + 512],
                                in_=ps[:],
                                func=AF.Relu,
                                bias=bc1_sb[:, ft : ft + 1],
                                scale=1.0,
                            )
                        units.append(u)
                return units

            def conv2(b, psG, ln2, hT, w2):
                for st in range(ST):
                    ps = psG.tile([128, D], F32, name="c2", tag="c2")
                    idx = 0
                    for k9 in range(K):
                        for fc in range(FT):
                            nc.tensor.matmul(
                                ps[:],
                                lhsT=hT[:, fc, st * 128 + k9 : st * 128 + k9 + 128],
                                rhs=w2[:, k9, fc, :],
                                start=(idx == 0),
                                stop=(idx == K * FT - 1),
                            )
                            idx += 1
                    t = ln2.tile([128, D], F32, tag="t")
                    nc.vector.tensor_add(out=t[:], in0=ps[:], in1=x1n[:, b, st, :])
                    nc.vector.tensor_add(out=t[:], in0=t[:], in1=bc2_sb)
                    stats = ln2.tile([128, 6], F32, tag="st")
                    nc.vector.bn_stats(out=stats[:], in_=t[:])
                    mv = ln2.tile([128, 2], F32, tag="mv")
                    nc.vector.bn_aggr(out=mv[:], in_=stats[:])
                    sd = ln2.tile([128, 1], F32, tag="sd")
                    nc.scalar.activation(
                        out=sd[:], in_=mv[:, 1:2], func=AF.Sqrt, bias=eps_sb[:],
                    )
                    nc.vector.reciprocal(sd[:], sd[:])
                    ot = ln2.tile([128, D], F32, tag="o")
                    nc.vector.tensor_scalar(
                        out=ot[:], in0=t[:], scalar1=mv[:, 0:1], scalar2=sd[:],
                        op0=ALU.subtract, op1=ALU.mult,
                    )
                    nc.vector.tensor_mul(out=ot[:], in0=ot[:], in1=g2_sb)
                    nc.vector.tensor_add(out=ot[:], in0=ot[:], in1=be2_sb)
                    nc.sync.dma_start(y_d[b, st], ot[:])

            # ---- phase 1: attention(b0), qkv(b1) woven in as PE filler ----
            with ExitStack() as p1:
                qkvp0 = p1.enter_context(tc.tile_pool(name="qkvp0", bufs=1))
                attn0 = p1.enter_context(tc.tile_pool(name="attn0", bufs=1))
                expp0 = p1.enter_context(tc.tile_pool(name="expp0", bufs=2))
                mskp0 = p1.enter_context(tc.tile_pool(name="mskp0", bufs=1))
                lnp0 = p1.enter_context(tc.tile_pool(name="lnp0", bufs=3))
                xnp0 = p1.enter_context(tc.tile_pool(name="xnp0", bufs=1))
                smal0 = p1.enter_context(tc.tile_pool(name="smal0", bufs=2))
                psA0 = p1.enter_context(tc.tile_pool(name="psA0", bufs=2, space="PSUM"))
                psB0 = p1.enter_context(tc.tile_pool(name="psB0", bufs=2, space="PSUM"))
                psC0 = p1.enter_context(tc.tile_pool(name="psC0", bufs=1, space="PSUM"))
                for u in qkv_units(0, qkvp0, psA0, smal0):
                    u()
                ua = attn_units(0, expp0, mskp0, smal0, lnp0, xnp0, psB0, psC0, attn0)
                ub = qkv_units(1, qkvp1, psA0, smal0)
                weave(ua, ub)

            # ---- phase 2: attention(b1) woven with conv1(b0) ----
            hT0p = ctx.enter_context(tc.tile_pool(name="hT0p", bufs=1, side="right"))
            hT0 = hT0p.tile([128, FT, SP], BF16, tag="hT0")
            nc.gpsimd.memset(hT0[:, :, 0:4], 0.0)
            nc.gpsimd.memset(hT0[:, :, 4 + S : SP], 0.0)
            w1p = ctx.enter_context(tc.tile_pool(name="w1p", bufs=2, side="right"))
            psF = ctx.enter_context(
                tc.tile_pool(name="psF", bufs=2, space="PSUM", side="right")
            )
            with ExitStack() as p2:
                attn1 = p2.enter_context(tc.tile_pool(name="attn1", bufs=1))
                expp1 = p2.enter_context(tc.tile_pool(name="expp1", bufs=1))
                mskp1 = p2.enter_context(tc.tile_pool(name="mskp1", bufs=1))
                lnp1 = p2.enter_context(tc.tile_pool(name="lnp1", bufs=3))
                xnp1 = p2.enter_context(tc.tile_pool(name="xnp1", bufs=1))
                smal1 = p2.enter_context(tc.tile_pool(name="smal1", bufs=2))
                psB1 = p2.enter_context(tc.tile_pool(name="psB1", bufs=2, space="PSUM"))
                psC1 = p2.enter_context(tc.tile_pool(name="psC1", bufs=1, space="PSUM"))
                ua = attn_units(1, expp1, mskp1, smal1, lnp1, xnp1, psB1, psC1, attn1)
                ub = conv1_units(0, w1p, psF, hT0)
                weave(ua, ub)

        # ---- phase 3: conv1(b1) + w2 chunks, then conv2(b0) ----
        with ExitStack() as p3:
            hT1p = p3.enter_context(tc.tile_pool(name="hT1p", bufs=1))
            hT1 = hT1p.tile([128, FT, SP], BF16, tag="hT1")
            nc.gpsimd.memset(hT1[:, :, 0:4], 0.0)
            nc.gpsimd.memset(hT1[:, :, 4 + S : SP], 0.0)
            w2p = p3.enter_context(tc.tile_pool(name="w2p", bufs=1))
            w2 = w2p.tile([128, K, FT, D], BF16, tag="w2")
            w2_dmas = [
                (lambda k9=k9: nc.sync.dma_start(w2[:, k9], wc2_d[:, k9]))
                for k9 in range(K)
            ]
            psG = p3.enter_context(tc.tile_pool(name="psG", bufs=4, space="PSUM"))
            ln2 = p3.enter_context(tc.tile_pool(name="ln2", bufs=2))
            for u in conv1_units(1, w1p, psF, hT1, extra_dma=w2_dmas):
                u()
            conv2(0, psG, ln2, hT0, w2)
            # ---- phase 4 ----
            conv2(1, psG, ln2, hT1, w2)


def _build():
    if "nc" not in _CACHE:
        nc = bacc.Bacc()
        _CACHE["nc"] = _emit(nc)
    return _CACHE["nc"]


def _prep_shared(Wq, bq, Wk, bk, Wv, bv, Wo, bo, Wc1, bc1, Wc2, bc2, g1, beta1, g2, beta2):
    bf = ml_dtypes.bfloat16
    f32 = np.float32
    sh = {}
    # [H, D, DH] -> [H, DC, 128, DH] -> [128, H, DC, DH]
    f8 = ml_dtypes.float8_e4m3
    # QKV weights in fp8, x512 (x1024 for V) to clear e4m3's subnormal floor;
    # the kernel divides the psum back down in the post-matmul activations.
    sh["wq"] = np.ascontiguousarray(
        np.clip(Wq * 512.0, -240, 240)
        .reshape(H, DC, 128, DH).transpose(2, 0, 1, 3).astype(f8))
    sh["wk"] = np.ascontiguousarray(
        np.clip(Wk * 512.0, -240, 240)
        .reshape(H, DC, 128, DH).transpose(2, 0, 1, 3).astype(f8))
    sh["wv"] = np.ascontiguousarray(
        np.clip(Wv * 1024.0, -240, 240)
        .reshape(H, DC, 128, DH).transpose(2, 0, 1, 3).astype(f8))
    # Wo in fp8, x64 (ON carries 16x -> psum = 1024*attn)
    wo = np.zeros((128, 4, D), dtype=f8)
    bounds = ((0, 128), (128, 192), (192, 320), (320, 384))
    for c, (r0, r1) in enumerate(bounds):
        wo[: r1 - r0, c] = np.clip(Wo[r0:r1] * 64.0, -240, 240).astype(f8)
    sh["wo"] = wo
    # [K, D, F] -> [FT, 128p(of D-chunk), K, DC, 128f]
    wc1 = Wc1.reshape(K, DC, 128, FT, 128)  # k, dc, p, ft, f
    sh["wc1"] = np.ascontiguousarray(wc1.transpose(3, 2, 0, 1, 4).astype(bf))
    # [K, F, D] -> [128p(of F-chunk), K, FT, D]
    wc2 = Wc2.reshape(K, FT, 128, D)
    sh["wc2"] = np.ascontiguousarray(wc2.transpose(2, 0, 1, 3).astype(bf))
    bqk = np.zeros((2, H, 2, 128), dtype=f32)
    for i, bb in enumerate((bq, bk)):
        for h in range(H):
            bqk[i, h, 0, :] = bb[h, :128] * 8.0
            bqk[i, h, 1, :64] = bb[h, 128:] * 8.0
    sh["bqk"] = np.ascontiguousarray(bqk.transpose(3, 0, 1, 2))
    sh["bv"] = bv.astype(f32) * 16.0
    sh["bc1t"] = np.ascontiguousarray(bc1.reshape(FT, 128).T.astype(f32))
    sh["lnc"] = np.ascontiguousarray(
        np.stack([g1, beta1, g2, beta2, bc2]).astype(f32))
    return sh


def run_sharded(inputs, trace=False):
    nc = _build()
    x = np.asarray(inputs["x"], dtype=np.float32)
    mask = np.asarray(inputs["mask"])
    sh = _prep_shared(
        *[np.asarray(inputs[k]) for k in (
            "Wq", "bq", "Wk", "bk", "Wv", "bv", "Wo", "bo",
            "Wc1", "bc1", "Wc2", "bc2", "g1", "beta1", "g2", "beta2",
        )]
    )
    bf = ml_dtypes.bfloat16
    bo = np.asarray(inputs["bo"], dtype=np.float32)
    in_maps = []
    for c in range(NCORES):
        xb = x[c * NB : (c + 1) * NB]  # [NB, S, D]
        m = {}
        # xT: [NB, 128p(of D-chunk), DC, S], fp8
        m["xT"] = np.ascontiguousarray(
            np.clip(xb.transpose(0, 2, 1), -240, 240)
            .reshape(NB, DC, 128, S).transpose(0, 2, 1, 3)
        ).astype(ml_dtypes.float8_e4m3)
        # xn: residual with bo folded in, x1024 to match the fp8 out-proj
        # psum scale (layernorm is scale-invariant); [NB, 128p, ST, D]
        m["xn"] = np.ascontiguousarray(
            ((xb + bo) * 1024.0).reshape(NB, ST, 128, D).transpose(0, 2, 1, 3)
        )
        mb = mask[c * NB : (c + 1) * NB]
        # mT: [NB, 128p(of k tile), ST, S_q], fp8 (0/1 exact)
        m["mT"] = np.ascontiguousarray(
            (~mb.transpose(0, 2, 1))
            .reshape(NB, ST, 128, S)
            .transpose(0, 2, 1, 3)
            .astype(ml_dtypes.float8_e4m3)
        )
        m.update(sh)
        in_maps.append(m)
    res = run_bass_kernel_spmd(nc, in_maps, core_ids=list(range(NCORES)), trace=trace)
    out = np.empty((B, S, D), dtype=np.float32)
    for c in range(NCORES):
        out[c * NB : (c + 1) * NB] = res.results[c]["y"].reshape(NB, S, D)
    return out, res


def kernel(**inputs):
    out, _ = run_sharded(inputs, trace=False)
    return out


# revision 54
# speedup vs baseline: 1.0158x; 1.0158x over previous
"""FFTBlock (attention + conv-FFN transformer block) on 8 Trainium2 NeuronCores.

Data-parallel over batch: 16 batch items -> 2 per core. Each core runs the
full block (MHA + LN + conv1d-FFN + LN) on its 2 batch items.

v2 changes over baseline:
  - All weights pre-transposed on host -> every weight DMA is a linear slab
    (the rearrange-DMAs were 384B-packet gathers that stalled startup).
  - Attention restructured as a kc-pipelined (flash-style) loop: per key tile
    scores -> exp -> mask -> A@V accumulate into persistent PSUM, with lag-2
    software pipelining so the PE never waits on the scalar/vector chain.
  - Mask loaded once per batch item (not per head), prefetched at phase start.
  - Residual (xn) tiles prefetched at phase start.
  - Out-projection/LN units decoupled from the x1T transposes (interleaved so
    the PE transpose never waits on the LN vector chain).
  - w2 (conv2 weights, 10.6MB) DMA split per-tap and interleaved with
    conv1(b1) so it no longer stalls the phase-3 boundary.
"""

import sys

sys.path.insert(0, "/opt/trn_rl_repo")

import math
from contextlib import ExitStack

import ml_dtypes
import numpy as np

import concourse.bass as bass
import concourse.mybir as mybir
import concourse.tile as tile
from concourse import bacc
from concourse.bass_utils import run_bass_kernel_spmd
from concourse.masks import make_identity

BF16 = mybir.dt.bfloat16
F32 = mybir.dt.float32
F8 = mybir.dt.float8e4
DR = mybir.MatmulPerfMode.DoubleRow
AF = mybir.ActivationFunctionType
ALU = mybir.AluOpType

B, S, D, H, DH, F, K = 16, 1024, 384, 2, 192, 1536, 9
NCORES = 8
NB = B // NCORES  # batch items per core
EPS = 1e-5
ISCALE = 1.0 / math.sqrt(D)  # NOTE: reference scales by sqrt(d_model)
SP = S + 8  # padded sequence length (4 left, 4 right)
DC = D // 128  # 3 d-chunks
FT = F // 128  # 12 filter tiles
ST = S // 128  # 8 seq tiles of 128
SQ = S // 512  # 2 seq chunks of 512

_CACHE = {}


def _bcast(ap, p=128):
    return bass.AP(tensor=ap.tensor, offset=ap.offset, ap=[[0, p]] + list(ap.ap))


def _emit(nc):
    # ---- DRAM I/O (all host-pretransposed: partition dim first) ----
    xT_d = nc.dram_tensor("xT", [NB, 128, DC, S], BF16, kind="ExternalInput")
    xn_d = nc.dram_tensor("xn", [NB, 128, ST, D], F32, kind="ExternalInput")
    mT_d = nc.dram_tensor("mT", [NB, 128, ST, S], F8, kind="ExternalInput")
    wq_d = nc.dram_tensor("wq", [128, H, DC, DH], BF16, kind="ExternalInput")
    wk_d = nc.dram_tensor("wk", [128, H, DC, DH], BF16, kind="ExternalInput")
    wv_d = nc.dram_tensor("wv", [128, H, DC, DH], BF16, kind="ExternalInput")
    wo_d = nc.dram_tensor("wo", [128, 4, D], F8, kind="ExternalInput")
    wc1_d = nc.dram_tensor("wc1", [FT, 128, K, DC, 128], BF16, kind="ExternalInput")
    wc2_d = nc.dram_tensor("wc2", [128, K, FT, D], BF16, kind="ExternalInput")
    bqk_d = nc.dram_tensor("bqk", [128, 2, H, 2], F32, kind="ExternalInput")
    bv_d = nc.dram_tensor("bv", [H, DH], F32, kind="ExternalInput")
    bc1_d = nc.dram_tensor("bc1t", [128, FT], F32, kind="ExternalInput")
    lnc_d = nc.dram_tensor("lnc", [5, D], F32, kind="ExternalInput")
    y_d = nc.dram_tensor("y", [NB, ST, 128, D], F32, kind="ExternalOutput")

    with tile.TileContext(nc) as tc:
        _body(nc, tc, locals())
    nc.finalize()
    return nc


def _body(nc, tc, d):
    xT_d, xn_d, mT_d = d["xT_d"], d["xn_d"], d["mT_d"]
    wq_d, wk_d, wv_d, wo_d = d["wq_d"], d["wk_d"], d["wv_d"], d["wo_d"]
    wc1_d, wc2_d = d["wc1_d"], d["wc2_d"]
    bqk_d, bv_d, bc1_d = d["bqk_d"], d["bv_d"], d["bc1_d"]
    lnc_d, y_d = d["lnc_d"], d["y_d"]

    with ExitStack() as ctx:
        const = ctx.enter_context(tc.tile_pool(name="const", bufs=1))
        persist = ctx.enter_context(tc.tile_pool(name="persist", bufs=1))

        # ---- critical-path weights first (all linear slabs now) ----
        wq_sb = const.tile([128, H, DC, DH], BF16, tag="wq")
        nc.sync.dma_start(wq_sb[:], wq_d[:])
        bqk_sb = const.tile([128, 2, H, 2], F32, tag="bqk")
        nc.sync.dma_start(bqk_sb[:], bqk_d[:])

        # ---- phased execution ----
        # P1: attention(b0) + qkv(b1) filler   P2: attention(b1) || conv1(b0)
        # P3: conv1(b1) || [w2 load chunks] then conv2(b0)   P4: conv2(b1)
        with ExitStack() as octx:
            qkvp1 = octx.enter_context(tc.tile_pool(name="qkvp1", bufs=1))
            xTp = octx.enter_context(tc.tile_pool(name="xTp", bufs=1))
            XT = {}
            for b in range(NB):
                XT[b] = xTp.tile([128, DC, S], BF16, name=f"xT{b}", tag=f"xT{b}")
                nc.sync.dma_start(XT[b][:], xT_d[b])
            wk_sb = const.tile([128, H, DC, DH], BF16, tag="wk")
            nc.sync.dma_start(wk_sb[:], wk_d[:])
            wv_sb = const.tile([128, H, DC, DH], BF16, tag="wv")
            nc.sync.dma_start(wv_sb[:], wv_d[:])
            bv_sb = const.tile([128, H, DH], F32, tag="bv")
            nc.sync.dma_start(bv_sb[:], _bcast(bv_d[:]))

            # remaining constants (off the critical path)
            wo_sb = const.tile([128, 4, D], F8, tag="wo")
            nc.sync.dma_start(wo_sb[:], wo_d[:])
            ident = const.tile([128, 128], F32, tag="ident")
            make_identity(nc, ident[:])
            bc1_sb = const.tile([128, FT], F32, tag="bc1")
            nc.sync.dma_start(bc1_sb[:], bc1_d[:])
            lnc_sb = const.tile([128, 5, D], F32, tag="lnc")
            nc.sync.dma_start(lnc_sb[:], _bcast(lnc_d[:]))
            g1_sb, be1_sb = lnc_sb[:, 0, :], lnc_sb[:, 1, :]
            g2_sb, be2_sb = lnc_sb[:, 2, :], lnc_sb[:, 3, :]
            bc2_sb = lnc_sb[:, 4, :]
            eps_sb = const.tile([128, 1], F32, tag="eps")
            nc.vector.memset(eps_sb[:], EPS)

            x1T = persist.tile([128, NB, DC, SP], BF16, tag="x1T")
            x1n = persist.tile([128, NB, ST, D], F32, tag="x1n")
            for b in range(NB):
                nc.gpsimd.memset(x1T[:, b, :, 0:4], 0.0)
                nc.gpsimd.memset(x1T[:, b, :, 4 + S : SP], 0.0)

            QT, KT, VV, ON = {}, {}, {}, {}

            def weave(a, b):
                # proportional merge of two unit lists; emits every closure
                ia = ib = 0
                while ia < len(a) or ib < len(b):
                    if ib >= len(b) or (ia < len(a) and ia * (len(b) + 1) <= ib * (len(a) + 1)):
                        a[ia](); ia += 1
                    else:
                        b[ib](); ib += 1

            def qkv_units(b, qkvp, psA, smal):
                units = []
                for h in range(H):
                    qk_us, v_us = [], []
                    # fp8 Q,K hold 8*(Q+bq); rows 64:128 of the second d-chunk
                    # are zeroed so DoubleRow's full-128 contraction adds 0.
                    qt = qkvp.tile([128, 2, S], F8, name=f"qt{b}{h}", tag=f"qt{b}{h}")
                    kt = qkvp.tile([128, 2, S], F8, name=f"kt{b}{h}", tag=f"kt{b}{h}")
                    # fp8 V holds 16*(V+bv) + a ones column; free dim padded to
                    # 208 so DoubleRow kc-pair strides are 16B-aligned.
                    vv = qkvp.tile([128, ST, 208], F8, name=f"vv{b}{h}", tag=f"vv{b}{h}")
                    QT[b, h], KT[b, h], VV[b, h] = qt, kt, vv
                    nc.gpsimd.memset(qt[64:128, 1, :], 0.0)
                    nc.gpsimd.memset(kt[64:128, 1, :], 0.0)
                    for wsb, bi, dst in ((wq_sb, 0, qt), (wk_sb, 1, kt)):
                        for mc, (m0, msz) in enumerate(((0, 128), (128, 64))):
                            for qc in range(SQ):
                                def u(b=b, h=h, wsb=wsb, bi=bi, dst=dst, m0=m0, msz=msz, mc=mc, qc=qc):
                                    ps = psA.tile([128, 512], F32, name="psqk", tag="p512")
                                    qs = slice(qc * 512, qc * 512 + 512)
                                    for dc in range(DC):
                                        nc.tensor.matmul(
                                            ps[:msz, :],
                                            lhsT=wsb[:, h, dc, m0 : m0 + msz],
                                            rhs=XT[b][:, dc, qs],
                                            start=(dc == 0),
                                            stop=(dc == DC - 1),
                                        )
                                    nc.scalar.activation(
                                        out=dst[:msz, mc, qc * 512 : qc * 512 + 512],
                                        in_=ps[:msz, :],
                                        func=AF.Identity,
                                        bias=bqk_sb[:msz, bi, h, mc : mc + 1],
                                        scale=8.0,
                                    )
                                qk_us.append(u)
                    for st in range(ST):
                        def u(b=b, h=h, vv=vv, st=st):
                            ps = psA.tile([128, 512], F32, name="psv", tag="p512")
                            ss = slice(st * 128, st * 128 + 128)
                            for dc in range(DC):
                                nc.tensor.matmul(
                                    ps[:, :DH],
                                    lhsT=XT[b][:, dc, ss],
                                    rhs=wv_sb[:, h, dc, :],
                                    start=(dc == 0),
                                    stop=(dc == DC - 1),
                                )
                            nc.vector.tensor_add(
                                out=vv[:, st, 0:DH], in0=ps[:, :DH], in1=bv_sb[:, h, :]
                            )
                            nc.gpsimd.memset(vv[:, st, DH : DH + 1], 1.0)
                        v_us.append(u)
                    units.extend(qk_us)
                    units.extend(v_us)
                return units

            def attn_units(b, expp, mskp, smal, lnp, xnp, psB, psC, attn):
                """kc-pipelined attention for item b.

                Per (h, kc, qc): scores matmul -> exp -> mask-mul, with the
                A@V accumulation for step i-2 emitted after step i's scores so
                the PE never waits on the scalar/vector chain (lag-2 pipeline).
                """
                units = []
                # mask + residual prefetch (one unit; DMAs overlap early scores)
                mask_sb = mskp.tile([128, ST, S], F8, name=f"msk{b}", tag=f"msk{b}")
                xn_sb = xnp.tile([128, ST, D], F32, name=f"xn{b}", tag=f"xn{b}")

                def u_prefetch(b=b, mask_sb=mask_sb, xn_sb=xn_sb):
                    nc.sync.dma_start(mask_sb[:], mT_d[b])
                    nc.sync.dma_start(xn_sb[:], xn_d[b])
                units.append(u_prefetch)

                # AV accumulators: 4 persistent PSUM banks (per qc: full + tail)
                acc = {}

                def sc_unit(b, h, kc, qc, expT):
                    qt, kt = QT[b, h], KT[b, h]
                    qs = slice(qc * 512, qc * 512 + 512)
                    ps = psB.tile([128, 512], F32, name="pssc", tag="sc")
                    # fp8 DoubleRow: both 96-deep halves (zero-padded to 128)
                    # in one pass; psum = 64 * scores.
                    nc.tensor.matmul(
                        ps[:, :],
                        lhsT=kt[:, :, kc * 128 : kc * 128 + 128],
                        rhs=qt[:, :, qs],
                        start=True,
                        stop=True,
                        perf_mode=DR,
                    )
                    nc.scalar.activation(
                        out=expT[:, kc, qs], in_=ps[:], func=AF.Exp, scale=ISCALE / 64.0,
                    )
                    nc.vector.tensor_mul(
                        out=expT[:, kc, qs], in0=expT[:, kc, qs], in1=mask_sb[:, kc, qs]
                    )

                def av_unit(b, h, kp, qc, expT):
                    # fp8 DoubleRow over a pair of key tiles (2kp, 2kp+1)
                    vv = VV[b, h]
                    qs = slice(qc * 512, qc * 512 + 512)
                    ps0, ps1 = acc[qc]
                    nc.tensor.matmul(
                        ps0[:],
                        lhsT=vv[:, 2 * kp : 2 * kp + 2, 0:128],
                        rhs=expT[:, 2 * kp : 2 * kp + 2, qs],
                        start=(kp == 0),
                        stop=(kp == ST // 2 - 1),
                        perf_mode=DR,
                    )
                    nc.tensor.matmul(
                        ps1[:65, :],
                        lhsT=vv[:, 2 * kp : 2 * kp + 2, 128 : DH + 1],
                        rhs=expT[:, 2 * kp : 2 * kp + 2, qs],
                        start=(kp == 0),
                        stop=(kp == ST // 2 - 1),
                        perf_mode=DR,
                    )

                for h in range(H):
                    expT = expp.tile([128, ST, S], F8, name=f"expT{h}", tag="expT")
                    onrm = attn.tile([128, 2, S], F8, name=f"on{b}{h}", tag=f"on{b}{h}")
                    ON[b, h] = onrm
                    nc.gpsimd.memset(onrm[64:128, 1, :], 0.0)

                    def u_alloc(h=h, b=b):
                        for qc in range(SQ):
                            ps0 = psC.tile([128, 512], F32, name=f"av0q{qc}", tag=f"av0q{qc}")
                            ps1 = psC.tile([128, 512], F32, name=f"av1q{qc}", tag=f"av1q{qc}")
                            acc[qc] = (ps0, ps1)

                    steps = [(kc, qc) for kc in range(ST) for qc in range(SQ)]
                    # av step (kp, qc) needs mask(2kp+1, qc) done, i.e. sc step
                    # index 2*(2kp+1)+qc; emit one sc later for pipeline slack.
                    av_after = {}
                    av_tail = []
                    for kp in range(ST // 2):
                        for qc in range(SQ):
                            gate = 2 * (2 * kp + 1) + qc + 1
                            if gate < len(steps):
                                av_after.setdefault(gate, []).append((kp, qc))
                            else:
                                av_tail.append((kp, qc))

                    def mk(i, h=h, expT=expT):
                        def u(i=i, h=h, expT=expT):
                            if i == 0:
                                u_alloc(h=h)
                            sc_unit(b, h, *steps[i], expT)
                            for kp, qc in av_after.get(i, ()):
                                av_unit(b, h, kp, qc, expT)
                        return u

                    for i in range(len(steps)):
                        units.append(mk(i))

                    def u_tail(b=b, h=h, expT=expT, onrm=onrm, av_tail=av_tail):
                        for kp, qc in av_tail:
                            av_unit(b, h, kp, qc, expT)
                        for qc in range(SQ):
                            qs = slice(qc * 512, qc * 512 + 512)
                            ps0, ps1 = acc[qc]
                            rc = smal.tile([1, 512], F32, tag="rc")
                            nc.scalar.copy(out=rc[:], in_=ps1[64:65, :])
                            rb = smal.tile([128, 512], F32, tag="rb")
                            nc.gpsimd.partition_broadcast(rb[:], rc[:])
                            nc.vector.reciprocal(rb[:], rb[:])
                            nc.vector.tensor_mul(out=onrm[:, 0, qs], in0=ps0[:], in1=rb[:])
                            nc.vector.tensor_mul(
                                out=onrm[:64, 1, qs], in0=ps1[:64, :], in1=rb[:64, :]
                            )
                    units.append(u_tail)

                # out-projection + LN (no transposes inside)
                def op_unit(st, b=b, xn_sb=xn_sb):
                    ps = psB.tile([128, 512], F32, name="at", tag="sc")
                    # fp8 DoubleRow per head; psum = 1024*attn, residual is
                    # host-prescaled by 1024 (layernorm is scale-invariant).
                    for h in range(H):
                        nc.tensor.matmul(
                            ps[:, :D],
                            lhsT=ON[b, h][:, :, st * 128 : st * 128 + 128],
                            rhs=wo_sb[:, 2 * h : 2 * h + 2, :],
                            start=(h == 0),
                            stop=(h == H - 1),
                            perf_mode=DR,
                        )
                    t = lnp.tile([128, D], F32, tag="t")
                    nc.vector.tensor_add(out=t[:], in0=ps[:, :D], in1=xn_sb[:, st, :])
                    stats = lnp.tile([128, 6], F32, tag="st")
                    nc.vector.bn_stats(out=stats[:], in_=t[:])
                    mv = lnp.tile([128, 2], F32, tag="mv")
                    nc.vector.bn_aggr(out=mv[:], in_=stats[:])
                    sd = lnp.tile([128, 1], F32, tag="sd")
                    nc.scalar.activation(
                        out=sd[:], in_=mv[:, 1:2], func=AF.Sqrt, bias=eps_sb[:],
                    )
                    nc.vector.reciprocal(sd[:], sd[:])
                    xv = x1n[:, b, st, :]
                    nc.vector.tensor_scalar(
                        out=xv, in0=t[:], scalar1=mv[:, 0:1], scalar2=sd[:],
                        op0=ALU.subtract, op1=ALU.mult,
                    )
                    nc.vector.tensor_mul(out=xv, in0=xv, in1=g1_sb)
                    nc.vector.tensor_add(out=xv, in0=xv, in1=be1_sb)

                def tr_unit(st, b=b):
                    for dc in range(DC):
                        tp = psB.tile([128, 512], F32, name="tp", tag="sc")
                        nc.tensor.transpose(
                            tp[:, :128], x1n[:, b, st, dc * 128 : dc * 128 + 128], ident[:]
                        )
                        nc.scalar.copy(
                            out=x1T[:, b, dc, 4 + st * 128 : 4 + st * 128 + 128],
                            in_=tp[:, :128],
                        )

                # transpose(st) runs three units after its LN chain, so the
                # PE never waits on the vector LN pipeline.
                LAG = 3
                sched = [("op", st) for st in range(LAG)]
                for st in range(LAG, ST):
                    sched += [("op", st), ("tr", st - LAG)]
                sched += [("tr", st) for st in range(ST - LAG, ST)]
                for kind, st in sched:
                    units.append(
                        (lambda st=st: op_unit(st)) if kind == "op"
                        else (lambda st=st: tr_unit(st))
                    )
                return units

            def conv1_units(b, w1p, psF, hT, extra_dma=None):
                units = []
                for ft in range(FT):
                    def udma(ft=ft):
                        w1 = w1p.tile([128, K, DC, 128], BF16, name="w1", tag="w1")
                        conv1_units._w1 = w1
                        nc.sync.dma_start(w1[:], wc1_d[ft])
                        if extra_dma is not None and ft < len(extra_dma):
                            extra_dma[ft]()
                    units.append(udma)
                    for qc in range(SQ):
                        def u(b=b, ft=ft, qc=qc):
                            w1 = conv1_units._w1
                            ps = psF.tile([128, 512], F32, name="c1", tag="c1")
                            idx = 0
                            for k9 in range(K):
                                for dc in range(DC):
                                    nc.tensor.matmul(
                                        ps[:],
                                        lhsT=w1[:, k9, dc, :],
                                        rhs=x1T[:, b, dc, qc * 512 + k9 : qc * 512 + k9 + 512],
                                        start=(idx == 0),
                                        stop=(idx == K * DC - 1),
                                    )
                                    idx += 1
                            nc.scalar.activation(
                                out=hT[:, ft, 4 + qc * 512 : 4 + qc * 512 + 512],
                                in_=ps[:],
                                func=AF.Relu,
                                bias=bc1_sb[:, ft : ft + 1],
                                scale=1.0,
                            )
                        units.append(u)
                return units

            def conv2(b, psG, ln2, hT, w2):
                for st in range(ST):
                    ps = psG.tile([128, D], F32, name="c2", tag="c2")
                    idx = 0
                    for k9 in range(K):
                        for fc in range(FT):
                            nc.tensor.matmul(
                                ps[:],
                                lhsT=hT[:, fc, st * 128 + k9 : st * 128 + k9 + 128],
                                rhs=w2[:, k9, fc, :],
                                start=(idx == 0),
                                stop=(idx == K * FT - 1),
                            )
                            idx += 1
                    t = ln2.tile([128, D], F32, tag="t")
                    nc.vector.tensor_add(out=t[:], in0=ps[:], in1=x1n[:, b, st, :])
                    nc.vector.tensor_add(out=t[:], in0=t[:], in1=bc2_sb)
                    stats = ln2.tile([128, 6], F32, tag="st")
                    nc.vector.bn_stats(out=stats[:], in_=t[:])
                    mv = ln2.tile([128, 2], F32, tag="mv")
                    nc.vector.bn_aggr(out=mv[:], in_=stats[:])
                    sd = ln2.tile([128, 1], F32, tag="sd")
                    nc.scalar.activation(
                        out=sd[:], in_=mv[:, 1:2], func=AF.Sqrt, bias=eps_sb[:],
                    )
                    nc.vector.reciprocal(sd[:], sd[:])
                    ot = ln2.tile([128, D], F32, tag="o")
                    nc.vector.tensor_scalar(
                        out=ot[:], in0=t[:], scalar1=mv[:, 0:1], scalar2=sd[:],
                        op0=ALU.subtract, op1=ALU.mult,
                    )
                    nc.vector.tensor_mul(out=ot[:], in0=ot[:], in1=g2_sb)
                    nc.vector.tensor_add(out=ot[:], in0=ot[:], in1=be2_sb)
                    nc.sync.dma_start(y_d[b, st], ot[:])

            # ---- phase 1: attention(b0), qkv(b1) woven in as PE filler ----
            with ExitStack() as p1:
                qkvp0 = p1.enter_context(tc.tile_pool(name="qkvp0", bufs=1))
                attn0 = p1.enter_context(tc.tile_pool(name="attn0", bufs=1))
                expp0 = p1.enter_context(tc.tile_pool(name="expp0", bufs=2))
                mskp0 = p1.enter_context(tc.tile_pool(name="mskp0", bufs=1))
                lnp0 = p1.enter_context(tc.tile_pool(name="lnp0", bufs=3))
                xnp0 = p1.enter_context(tc.tile_pool(name="xnp0", bufs=1))
                smal0 = p1.enter_context(tc.tile_pool(name="smal0", bufs=2))
                psA0 = p1.enter_context(tc.tile_pool(name="psA0", bufs=2, space="PSUM"))
                psB0 = p1.enter_context(tc.tile_pool(name="psB0", bufs=2, space="PSUM"))
                psC0 = p1.enter_context(tc.tile_pool(name="psC0", bufs=1, space="PSUM"))
                for u in qkv_units(0, qkvp0, psA0, smal0):
                    u()
                ua = attn_units(0, expp0, mskp0, smal0, lnp0, xnp0, psB0, psC0, attn0)
                ub = qkv_units(1, qkvp1, psA0, smal0)
                weave(ua, ub)

            # ---- phase 2: attention(b1) woven with conv1(b0) ----
            hT0p = ctx.enter_context(tc.tile_pool(name="hT0p", bufs=1, side="right"))
            hT0 = hT0p.tile([128, FT, SP], BF16, tag="hT0")
            nc.gpsimd.memset(hT0[:, :, 0:4], 0.0)
            nc.gpsimd.memset(hT0[:, :, 4 + S : SP], 0.0)
            w1p = ctx.enter_context(tc.tile_pool(name="w1p", bufs=2, side="right"))
            psF = ctx.enter_context(
                tc.tile_pool(name="psF", bufs=2, space="PSUM", side="right")
            )
            with ExitStack() as p2:
                attn1 = p2.enter_context(tc.tile_pool(name="attn1", bufs=1))
                expp1 = p2.enter_context(tc.tile_pool(name="expp1", bufs=1))
                mskp1 = p2.enter_context(tc.tile_pool(name="mskp1", bufs=1))
                lnp1 = p2.enter_context(tc.tile_pool(name="lnp1", bufs=3))
                xnp1 = p2.enter_context(tc.tile_pool(name="xnp1", bufs=1))
                smal1 = p2.enter_context(tc.tile_pool(name="smal1", bufs=2))
                psB1 = p2.enter_context(tc.tile_pool(name="psB1", bufs=2, space="PSUM"))
                psC1 = p2.enter_context(tc.tile_pool(name="psC1", bufs=1, space="PSUM"))
                ua = attn_units(1, expp1, mskp1, smal1, lnp1, xnp1, psB1, psC1, attn1)
                ub = conv1_units(0, w1p, psF, hT0)
                weave(ua, ub)

        # ---- phase 3: conv1(b1) + w2 chunks, then conv2(b0) ----
        with ExitStack() as p3:
            hT1p = p3.enter_context(tc.tile_pool(name="hT1p", bufs=1))
            hT1 = hT1p.tile([128, FT, SP], BF16, tag="hT1")
            nc.gpsimd.memset(hT1[:, :, 0:4], 0.0)
            nc.gpsimd.memset(hT1[:, :, 4 + S : SP], 0.0)
            w2p = p3.enter_context(tc.tile_pool(name="w2p", bufs=1))
            w2 = w2p.tile([128, K, FT, D], BF16, tag="w2")
            w2_dmas = [
                (lambda k9=k9: nc.sync.dma_start(w2[:, k9], wc2_d[:, k9]))
                for k9 in range(K)
            ]
            psG = p3.enter_context(tc.tile_pool(name="psG", bufs=4, space="PSUM"))
            ln2 = p3.enter_context(tc.tile_pool(name="ln2", bufs=2))
            for u in conv1_units(1, w1p, psF, hT1, extra_dma=w2_dmas):
                u()
            conv2(0, psG, ln2, hT0, w2)
            # ---- phase 4 ----
            conv2(1, psG, ln2, hT1, w2)


def _build():
    if "nc" not in _CACHE:
        nc = bacc.Bacc()
        _CACHE["nc"] = _emit(nc)
    return _CACHE["nc"]


def _prep_shared(Wq, bq, Wk, bk, Wv, bv, Wo, bo, Wc1, bc1, Wc2, bc2, g1, beta1, g2, beta2):
    bf = ml_dtypes.bfloat16
    f32 = np.float32
    sh = {}
    # [H, D, DH] -> [H, DC, 128, DH] -> [128, H, DC, DH]
    f8 = ml_dtypes.float8_e4m3
    sh["wq"] = np.ascontiguousarray(
        Wq.reshape(H, DC, 128, DH).transpose(2, 0, 1, 3).astype(bf))
    sh["wk"] = np.ascontiguousarray(
        Wk.reshape(H, DC, 128, DH).transpose(2, 0, 1, 3).astype(bf))
    # V path carries 16x so the fp8 vv tile is well inside e4m3's range
    sh["wv"] = np.ascontiguousarray(
        (Wv * 16.0).reshape(H, DC, 128, DH).transpose(2, 0, 1, 3).astype(bf))
    # Wo in fp8, x64 (ON carries 16x -> psum = 1024*attn)
    wo = np.zeros((128, 4, D), dtype=f8)
    bounds = ((0, 128), (128, 192), (192, 320), (320, 384))
    for c, (r0, r1) in enumerate(bounds):
        wo[: r1 - r0, c] = np.clip(Wo[r0:r1] * 64.0, -240, 240).astype(f8)
    sh["wo"] = wo
    # [K, D, F] -> [FT, 128p(of D-chunk), K, DC, 128f]
    wc1 = Wc1.reshape(K, DC, 128, FT, 128)  # k, dc, p, ft, f
    sh["wc1"] = np.ascontiguousarray(wc1.transpose(3, 2, 0, 1, 4).astype(bf))
    # [K, F, D] -> [128p(of F-chunk), K, FT, D]
    wc2 = Wc2.reshape(K, FT, 128, D)
    sh["wc2"] = np.ascontiguousarray(wc2.transpose(2, 0, 1, 3).astype(bf))
    bqk = np.zeros((2, H, 2, 128), dtype=f32)
    for i, bb in enumerate((bq, bk)):
        for h in range(H):
            bqk[i, h, 0, :] = bb[h, :128] * 8.0
            bqk[i, h, 1, :64] = bb[h, 128:] * 8.0
    sh["bqk"] = np.ascontiguousarray(bqk.transpose(3, 0, 1, 2))
    sh["bv"] = bv.astype(f32) * 16.0
    sh["bc1t"] = np.ascontiguousarray(bc1.reshape(FT, 128).T.astype(f32))
    sh["lnc"] = np.ascontiguousarray(
        np.stack([g1, beta1, g2, beta2, bc2]).astype(f32))
    return sh


def run_sharded(inputs, trace=False):
    nc = _build()
    x = np.asarray(inputs["x"], dtype=np.float32)
    mask = np.asarray(inputs["mask"])
    sh = _prep_shared(
        *[np.asarray(inputs[k]) for k in (
            "Wq", "bq", "Wk", "bk", "Wv", "bv", "Wo", "bo",
            "Wc1", "bc1", "Wc2", "bc2", "g1", "beta1", "g2", "beta2",
        )]
    )
    bf = ml_dtypes.bfloat16
    bo = np.asarray(inputs["bo"], dtype=np.float32)
    in_maps = []
    for c in range(NCORES):
        xb = x[c * NB : (c + 1) * NB]  # [NB, S, D]
        m = {}
        # xT: [NB, 128p(of D-chunk), DC, S]
        m["xT"] = np.ascontiguousarray(
            xb.transpose(0, 2, 1).reshape(NB, DC, 128, S).transpose(0, 2, 1, 3)
        ).astype(bf)
        # xn: residual with bo folded in, x1024 to match the fp8 out-proj
        # psum scale (layernorm is scale-invariant); [NB, 128p, ST, D]
        m["xn"] = np.ascontiguousarray(
            ((xb + bo) * 1024.0).reshape(NB, ST, 128, D).transpose(0, 2, 1, 3)
        )
        mb = mask[c * NB : (c + 1) * NB]
        # mT: [NB, 128p(of k tile), ST, S_q], fp8 (0/1 exact)
        m["mT"] = np.ascontiguousarray(
            (~mb.transpose(0, 2, 1))
            .reshape(NB, ST, 128, S)
            .transpose(0, 2, 1, 3)
            .astype(ml_dtypes.float8_e4m3)
        )
        m.update(sh)
        in_maps.append(m)
    res = run_bass_kernel_spmd(nc, in_maps, core_ids=list(range(NCORES)), trace=trace)
    out = np.empty((B, S, D), dtype=np.float32)
    for c in range(NCORES):
        out[c * NB : (c + 1) * NB] = res.results[c]["y"].reshape(NB, S, D)
    return out, res


def kernel(**inputs):
    out, _ = run_sharded(inputs, trace=False)
    return out


# revision 55
# speedup vs baseline: 1.0196x; 1.0037x over previous
"""FFTBlock (attention + conv-FFN transformer block) on 8 Trainium2 NeuronCores.

Data-parallel over batch: 16 batch items -> 2 per core. Each core runs the
full block (MHA + LN + conv1d-FFN + LN) on its 2 batch items.

v2 changes over baseline:
  - All weights pre-transposed on host -> every weight DMA is a linear slab
    (the rearrange-DMAs were 384B-packet gathers that stalled startup).
  - Attention restructured as a kc-pipelined (flash-style) loop: per key tile
    scores -> exp -> mask -> A@V accumulate into persistent PSUM, with lag-2
    software pipelining so the PE never waits on the scalar/vector chain.
  - Mask loaded once per batch item (not per head), prefetched at phase start.
  - Residual (xn) tiles prefetched at phase start.
  - Out-projection/LN units decoupled from the x1T transposes (interleaved so
    the PE transpose never waits on the LN vector chain).
  - w2 (conv2 weights, 10.6MB) DMA split per-tap and interleaved with
    conv1(b1) so it no longer stalls the phase-3 boundary.
"""

import sys

sys.path.insert(0, "/opt/trn_rl_repo")

import math
from contextlib import ExitStack

import ml_dtypes
import numpy as np

import concourse.bass as bass
import concourse.mybir as mybir
import concourse.tile as tile
from concourse import bacc
from concourse.bass_utils import run_bass_kernel_spmd
from concourse.masks import make_identity

BF16 = mybir.dt.bfloat16
F32 = mybir.dt.float32
F8 = mybir.dt.float8e4
DR = mybir.MatmulPerfMode.DoubleRow
AF = mybir.ActivationFunctionType
ALU = mybir.AluOpType

B, S, D, H, DH, F, K = 16, 1024, 384, 2, 192, 1536, 9
NCORES = 8
NB = B // NCORES  # batch items per core
EPS = 1e-5
ISCALE = 1.0 / math.sqrt(D)  # NOTE: reference scales by sqrt(d_model)
SP = S + 8  # padded sequence length (4 left, 4 right)
DC = D // 128  # 3 d-chunks
FT = F // 128  # 12 filter tiles
ST = S // 128  # 8 seq tiles of 128
SQ = S // 512  # 2 seq chunks of 512

_CACHE = {}


def _bcast(ap, p=128):
    return bass.AP(tensor=ap.tensor, offset=ap.offset, ap=[[0, p]] + list(ap.ap))


def _emit(nc):
    # ---- DRAM I/O (all host-pretransposed: partition dim first) ----
    xT_d = nc.dram_tensor("xT", [NB, 128, DC, S], BF16, kind="ExternalInput")
    xn_d = nc.dram_tensor("xn", [NB, 128, ST, D], F32, kind="ExternalInput")
    mT_d = nc.dram_tensor("mT", [NB, 128, ST, S], F8, kind="ExternalInput")
    wq_d = nc.dram_tensor("wq", [128, H, DC, DH], BF16, kind="ExternalInput")
    wk_d = nc.dram_tensor("wk", [128, H, DC, DH], BF16, kind="ExternalInput")
    wv_d = nc.dram_tensor("wv", [128, H, DC, DH], BF16, kind="ExternalInput")
    wo_d = nc.dram_tensor("wo", [128, 4, D], F8, kind="ExternalInput")
    wc1_d = nc.dram_tensor("wc1", [FT, 128, K, DC, 128], BF16, kind="ExternalInput")
    wc2_d = nc.dram_tensor("wc2", [128, K, FT, D], BF16, kind="ExternalInput")
    bqk_d = nc.dram_tensor("bqk", [128, 2, H, 2], F32, kind="ExternalInput")
    bv_d = nc.dram_tensor("bv", [H, DH], F32, kind="ExternalInput")
    bc1_d = nc.dram_tensor("bc1t", [128, FT], F32, kind="ExternalInput")
    lnc_d = nc.dram_tensor("lnc", [5, D], F32, kind="ExternalInput")
    y_d = nc.dram_tensor("y", [NB, ST, 128, D], F32, kind="ExternalOutput")

    with tile.TileContext(nc) as tc:
        _body(nc, tc, locals())
    nc.finalize()
    return nc


def _body(nc, tc, d):
    xT_d, xn_d, mT_d = d["xT_d"], d["xn_d"], d["mT_d"]
    wq_d, wk_d, wv_d, wo_d = d["wq_d"], d["wk_d"], d["wv_d"], d["wo_d"]
    wc1_d, wc2_d = d["wc1_d"], d["wc2_d"]
    bqk_d, bv_d, bc1_d = d["bqk_d"], d["bv_d"], d["bc1_d"]
    lnc_d, y_d = d["lnc_d"], d["y_d"]

    with ExitStack() as ctx:
        const = ctx.enter_context(tc.tile_pool(name="const", bufs=1))
        persist = ctx.enter_context(tc.tile_pool(name="persist", bufs=1))

        # ---- critical-path weights first (all linear slabs now) ----
        wq_sb = const.tile([128, H, DC, DH], BF16, tag="wq")
        nc.sync.dma_start(wq_sb[:], wq_d[:])
        bqk_sb = const.tile([128, 2, H, 2], F32, tag="bqk")
        nc.sync.dma_start(bqk_sb[:], bqk_d[:])

        # ---- phased execution ----
        # P1: attention(b0) + qkv(b1) filler   P2: attention(b1) || conv1(b0)
        # P3: conv1(b1) || [w2 load chunks] then conv2(b0)   P4: conv2(b1)
        with ExitStack() as octx:
            qkvp1 = octx.enter_context(tc.tile_pool(name="qkvp1", bufs=1))
            xTp = octx.enter_context(tc.tile_pool(name="xTp", bufs=1))
            XT = {}
            for b in range(NB):
                XT[b] = xTp.tile([128, DC, S], BF16, name=f"xT{b}", tag=f"xT{b}")
                nc.sync.dma_start(XT[b][:], xT_d[b])
            wk_sb = const.tile([128, H, DC, DH], BF16, tag="wk")
            nc.sync.dma_start(wk_sb[:], wk_d[:])
            wv_sb = const.tile([128, H, DC, DH], BF16, tag="wv")
            nc.sync.dma_start(wv_sb[:], wv_d[:])
            bv_sb = const.tile([128, H, DH], F32, tag="bv")
            nc.sync.dma_start(bv_sb[:], _bcast(bv_d[:]))

            # remaining constants (off the critical path)
            wo_sb = const.tile([128, 4, D], F8, tag="wo")
            nc.sync.dma_start(wo_sb[:], wo_d[:])
            ident = const.tile([128, 128], F32, tag="ident")
            make_identity(nc, ident[:])
            bc1_sb = const.tile([128, FT], F32, tag="bc1")
            nc.sync.dma_start(bc1_sb[:], bc1_d[:])
            lnc_sb = const.tile([128, 5, D], F32, tag="lnc")
            nc.sync.dma_start(lnc_sb[:], _bcast(lnc_d[:]))
            g1_sb, be1_sb = lnc_sb[:, 0, :], lnc_sb[:, 1, :]
            g2_sb, be2_sb = lnc_sb[:, 2, :], lnc_sb[:, 3, :]
            bc2_sb = lnc_sb[:, 4, :]
            eps_sb = const.tile([128, 1], F32, tag="eps")
            nc.vector.memset(eps_sb[:], EPS)

            x1T = persist.tile([128, NB, DC, SP], BF16, tag="x1T")
            x1n = persist.tile([128, NB, ST, D], F32, tag="x1n")
            for b in range(NB):
                nc.gpsimd.memset(x1T[:, b, :, 0:4], 0.0)
                nc.gpsimd.memset(x1T[:, b, :, 4 + S : SP], 0.0)

            QT, KT, VV, ON = {}, {}, {}, {}

            def weave(a, b):
                # proportional merge of two unit lists; emits every closure
                ia = ib = 0
                while ia < len(a) or ib < len(b):
                    if ib >= len(b) or (ia < len(a) and ia * (len(b) + 1) <= ib * (len(a) + 1)):
                        a[ia](); ia += 1
                    else:
                        b[ib](); ib += 1

            def qkv_units(b, qkvp, psA, smal):
                units = []
                for h in range(H):
                    qk_us, v_us = [], []
                    # fp8 Q,K hold 8*(Q+bq); rows 64:128 of the second d-chunk
                    # are zeroed so DoubleRow's full-128 contraction adds 0.
                    qt = qkvp.tile([128, 2, S], F8, name=f"qt{b}{h}", tag=f"qt{b}{h}")
                    kt = qkvp.tile([128, 2, S], F8, name=f"kt{b}{h}", tag=f"kt{b}{h}")
                    # fp8 V holds 16*(V+bv) + a ones column; free dim padded to
                    # 208 so DoubleRow kc-pair strides are 16B-aligned.
                    vv = qkvp.tile([128, ST, 208], F8, name=f"vv{b}{h}", tag=f"vv{b}{h}")
                    QT[b, h], KT[b, h], VV[b, h] = qt, kt, vv
                    nc.gpsimd.memset(qt[64:128, 1, :], 0.0)
                    nc.gpsimd.memset(kt[64:128, 1, :], 0.0)
                    for wsb, bi, dst in ((wq_sb, 0, qt), (wk_sb, 1, kt)):
                        for mc, (m0, msz) in enumerate(((0, 128), (128, 64))):
                            for qc in range(SQ):
                                def u(b=b, h=h, wsb=wsb, bi=bi, dst=dst, m0=m0, msz=msz, mc=mc, qc=qc):
                                    ps = psA.tile([128, 512], F32, name="psqk", tag="p512")
                                    qs = slice(qc * 512, qc * 512 + 512)
                                    for dc in range(DC):
                                        nc.tensor.matmul(
                                            ps[:msz, :],
                                            lhsT=wsb[:, h, dc, m0 : m0 + msz],
                                            rhs=XT[b][:, dc, qs],
                                            start=(dc == 0),
                                            stop=(dc == DC - 1),
                                        )
                                    nc.scalar.activation(
                                        out=dst[:msz, mc, qc * 512 : qc * 512 + 512],
                                        in_=ps[:msz, :],
                                        func=AF.Identity,
                                        bias=bqk_sb[:msz, bi, h, mc : mc + 1],
                                        scale=8.0,
                                    )
                                qk_us.append(u)
                    for st in range(ST):
                        def u(b=b, h=h, vv=vv, st=st):
                            ps = psA.tile([128, 512], F32, name="psv", tag="p512")
                            ss = slice(st * 128, st * 128 + 128)
                            for dc in range(DC):
                                nc.tensor.matmul(
                                    ps[:, :DH],
                                    lhsT=XT[b][:, dc, ss],
                                    rhs=wv_sb[:, h, dc, :],
                                    start=(dc == 0),
                                    stop=(dc == DC - 1),
                                )
                            nc.vector.tensor_add(
                                out=vv[:, st, 0:DH], in0=ps[:, :DH], in1=bv_sb[:, h, :]
                            )
                            nc.gpsimd.memset(vv[:, st, DH : DH + 1], 1.0)
                        v_us.append(u)
                    units.extend(qk_us)
                    units.extend(v_us)
                return units

            def attn_units(b, expp, mskp, smal, lnp, xnp, psB, psC, attn):
                """kc-pipelined attention for item b.

                Per (h, kc, qc): scores matmul -> exp -> mask-mul, with the
                A@V accumulation for step i-2 emitted after step i's scores so
                the PE never waits on the scalar/vector chain (lag-2 pipeline).
                """
                units = []
                # mask + residual prefetch (one unit; DMAs overlap early scores)
                mask_sb = mskp.tile([128, ST, S], F8, name=f"msk{b}", tag=f"msk{b}")
                xn_sb = xnp.tile([128, ST, D], F32, name=f"xn{b}", tag=f"xn{b}")

                def u_prefetch(b=b, mask_sb=mask_sb, xn_sb=xn_sb):
                    nc.sync.dma_start(mask_sb[:], mT_d[b])
                    nc.sync.dma_start(xn_sb[:], xn_d[b])
                units.append(u_prefetch)

                # AV accumulators: 4 persistent PSUM banks (per qc: full + tail)
                acc = {}

                def sc_unit(b, h, kc, qc, expT):
                    qt, kt = QT[b, h], KT[b, h]
                    qs = slice(qc * 512, qc * 512 + 512)
                    ps = psB.tile([128, 512], F32, name="pssc", tag="sc")
                    # fp8 DoubleRow: both 96-deep halves (zero-padded to 128)
                    # in one pass; psum = 64 * scores.
                    nc.tensor.matmul(
                        ps[:, :],
                        lhsT=kt[:, :, kc * 128 : kc * 128 + 128],
                        rhs=qt[:, :, qs],
                        start=True,
                        stop=True,
                        perf_mode=DR,
                    )
                    nc.scalar.activation(
                        out=expT[:, kc, qs], in_=ps[:], func=AF.Exp, scale=ISCALE / 64.0,
                    )
                    nc.vector.tensor_mul(
                        out=expT[:, kc, qs], in0=expT[:, kc, qs], in1=mask_sb[:, kc, qs]
                    )

                def av_unit(b, h, kp, qc, expT):
                    # fp8 DoubleRow over a pair of key tiles (2kp, 2kp+1)
                    vv = VV[b, h]
                    qs = slice(qc * 512, qc * 512 + 512)
                    ps0, ps1 = acc[qc]
                    nc.tensor.matmul(
                        ps0[:],
                        lhsT=vv[:, 2 * kp : 2 * kp + 2, 0:128],
                        rhs=expT[:, 2 * kp : 2 * kp + 2, qs],
                        start=(kp == 0),
                        stop=(kp == ST // 2 - 1),
                        perf_mode=DR,
                    )
                    nc.tensor.matmul(
                        ps1[:65, :],
                        lhsT=vv[:, 2 * kp : 2 * kp + 2, 128 : DH + 1],
                        rhs=expT[:, 2 * kp : 2 * kp + 2, qs],
                        start=(kp == 0),
                        stop=(kp == ST // 2 - 1),
                        perf_mode=DR,
                    )

                for h in range(H):
                    expT = expp.tile([128, ST, S], F8, name=f"expT{h}", tag="expT")
                    onrm = attn.tile([128, 2, S], F8, name=f"on{b}{h}", tag=f"on{b}{h}")
                    ON[b, h] = onrm
                    nc.gpsimd.memset(onrm[64:128, 1, :], 0.0)

                    def u_alloc(h=h, b=b):
                        for qc in range(SQ):
                            ps0 = psC.tile([128, 512], F32, name=f"av0q{qc}", tag=f"av0q{qc}")
                            ps1 = psC.tile([128, 512], F32, name=f"av1q{qc}", tag=f"av1q{qc}")
                            acc[qc] = (ps0, ps1)

                    steps = [(kc, qc) for kc in range(ST) for qc in range(SQ)]
                    # av step (kp, qc) needs mask(2kp+1, qc) done, i.e. sc step
                    # index 2*(2kp+1)+qc; emit one sc later for pipeline slack.
                    av_after = {}
                    av_tail = []
                    for kp in range(ST // 2):
                        for qc in range(SQ):
                            gate = 2 * (2 * kp + 1) + qc + 2
                            if gate < len(steps):
                                av_after.setdefault(gate, []).append((kp, qc))
                            else:
                                av_tail.append((kp, qc))

                    def mk(i, h=h, expT=expT):
                        def u(i=i, h=h, expT=expT):
                            if i == 0:
                                u_alloc(h=h)
                            sc_unit(b, h, *steps[i], expT)
                            for kp, qc in av_after.get(i, ()):
                                av_unit(b, h, kp, qc, expT)
                        return u

                    for i in range(len(steps)):
                        units.append(mk(i))

                    def u_tail(b=b, h=h, expT=expT, onrm=onrm, av_tail=av_tail):
                        for kp, qc in av_tail:
                            av_unit(b, h, kp, qc, expT)
                        for qc in range(SQ):
                            qs = slice(qc * 512, qc * 512 + 512)
                            ps0, ps1 = acc[qc]
                            rc = smal.tile([1, 512], F32, tag="rc")
                            nc.scalar.copy(out=rc[:], in_=ps1[64:65, :])
                            rb = smal.tile([128, 512], F32, tag="rb")
                            nc.gpsimd.partition_broadcast(rb[:], rc[:])
                            nc.vector.reciprocal(rb[:], rb[:])
                            nc.vector.tensor_mul(out=onrm[:, 0, qs], in0=ps0[:], in1=rb[:])
                            nc.vector.tensor_mul(
                                out=onrm[:64, 1, qs], in0=ps1[:64, :], in1=rb[:64, :]
                            )
                    units.append(u_tail)

                # out-projection + LN (no transposes inside)
                def op_unit(st, b=b, xn_sb=xn_sb):
                    ps = psB.tile([128, 512], F32, name="at", tag="sc")
                    # fp8 DoubleRow per head; psum = 1024*attn, residual is
                    # host-prescaled by 1024 (layernorm is scale-invariant).
                    for h in range(H):
                        nc.tensor.matmul(
                            ps[:, :D],
                            lhsT=ON[b, h][:, :, st * 128 : st * 128 + 128],
                            rhs=wo_sb[:, 2 * h : 2 * h + 2, :],
                            start=(h == 0),
                            stop=(h == H - 1),
                            perf_mode=DR,
                        )
                    t = lnp.tile([128, D], F32, tag="t")
                    nc.vector.tensor_add(out=t[:], in0=ps[:, :D], in1=xn_sb[:, st, :])
                    stats = lnp.tile([128, 6], F32, tag="st")
                    nc.vector.bn_stats(out=stats[:], in_=t[:])
                    mv = lnp.tile([128, 2], F32, tag="mv")
                    nc.vector.bn_aggr(out=mv[:], in_=stats[:])
                    sd = lnp.tile([128, 1], F32, tag="sd")
                    nc.scalar.activation(
                        out=sd[:], in_=mv[:, 1:2], func=AF.Sqrt, bias=eps_sb[:],
                    )
                    nc.vector.reciprocal(sd[:], sd[:])
                    xv = x1n[:, b, st, :]
                    nc.vector.tensor_scalar(
                        out=xv, in0=t[:], scalar1=mv[:, 0:1], scalar2=sd[:],
                        op0=ALU.subtract, op1=ALU.mult,
                    )
                    nc.vector.tensor_mul(out=xv, in0=xv, in1=g1_sb)
                    nc.vector.tensor_add(out=xv, in0=xv, in1=be1_sb)

                def tr_unit(st, b=b):
                    for dc in range(DC):
                        tp = psB.tile([128, 512], F32, name="tp", tag="sc")
                        nc.tensor.transpose(
                            tp[:, :128], x1n[:, b, st, dc * 128 : dc * 128 + 128], ident[:]
                        )
                        nc.scalar.copy(
                            out=x1T[:, b, dc, 4 + st * 128 : 4 + st * 128 + 128],
                            in_=tp[:, :128],
                        )

                # transpose(st) runs three units after its LN chain, so the
                # PE never waits on the vector LN pipeline.
                LAG = 3
                sched = [("op", st) for st in range(LAG)]
                for st in range(LAG, ST):
                    sched += [("op", st), ("tr", st - LAG)]
                sched += [("tr", st) for st in range(ST - LAG, ST)]
                for kind, st in sched:
                    units.append(
                        (lambda st=st: op_unit(st)) if kind == "op"
                        else (lambda st=st: tr_unit(st))
                    )
                return units

            def conv1_units(b, w1p, psF, hT, extra_dma=None):
                units = []
                for ft in range(FT):
                    def udma(ft=ft):
                        w1 = w1p.tile([128, K, DC, 128], BF16, name="w1", tag="w1")
                        conv1_units._w1 = w1
                        nc.sync.dma_start(w1[:], wc1_d[ft])
                        if extra_dma is not None and ft < len(extra_dma):
                            extra_dma[ft]()
                    units.append(udma)
                    for qc in range(SQ):
                        def u(b=b, ft=ft, qc=qc):
                            w1 = conv1_units._w1
                            ps = psF.tile([128, 512], F32, name="c1", tag="c1")
                            idx = 0
                            for k9 in range(K):
                                for dc in range(DC):
                                    nc.tensor.matmul(
                                        ps[:],
                                        lhsT=w1[:, k9, dc, :],
                                        rhs=x1T[:, b, dc, qc * 512 + k9 : qc * 512 + k9 + 512],
                                        start=(idx == 0),
                                        stop=(idx == K * DC - 1),
                                    )
                                    idx += 1
                            nc.scalar.activation(
                                out=hT[:, ft, 4 + qc * 512 : 4 + qc * 512 + 512],
                                in_=ps[:],
                                func=AF.Relu,
                                bias=bc1_sb[:, ft : ft + 1],
                                scale=1.0,
                            )
                        units.append(u)
                return units

            def conv2(b, psG, ln2, hT, w2):
                for st in range(ST):
                    ps = psG.tile([128, D], F32, name="c2", tag="c2")
                    idx = 0
                    for k9 in range(K):
                        for fc in range(FT):
                            nc.tensor.matmul(
                                ps[:],
                                lhsT=hT[:, fc, st * 128 + k9 : st * 128 + k9 + 128],
                                rhs=w2[:, k9, fc, :],
                                start=(idx == 0),
                                stop=(idx == K * FT - 1),
                            )
                            idx += 1
                    t = ln2.tile([128, D], F32, tag="t")
                    nc.vector.tensor_add(out=t[:], in0=ps[:], in1=x1n[:, b, st, :])
                    nc.vector.tensor_add(out=t[:], in0=t[:], in1=bc2_sb)
                    stats = ln2.tile([128, 6], F32, tag="st")
                    nc.vector.bn_stats(out=stats[:], in_=t[:])
                    mv = ln2.tile([128, 2], F32, tag="mv")
                    nc.vector.bn_aggr(out=mv[:], in_=stats[:])
                    sd = ln2.tile([128, 1], F32, tag="sd")
                    nc.scalar.activation(
                        out=sd[:], in_=mv[:, 1:2], func=AF.Sqrt, bias=eps_sb[:],
                    )
                    nc.vector.reciprocal(sd[:], sd[:])
                    ot = ln2.tile([128, D], F32, tag="o")
                    nc.vector.tensor_scalar(
                        out=ot[:], in0=t[:], scalar1=mv[:, 0:1], scalar2=sd[:],
                        op0=ALU.subtract, op1=ALU.mult,
                    )
                    nc.vector.tensor_mul(out=ot[:], in0=ot[:], in1=g2_sb)
                    nc.vector.tensor_add(out=ot[:], in0=ot[:], in1=be2_sb)
                    nc.sync.dma_start(y_d[b, st], ot[:])

            # ---- phase 1: attention(b0), qkv(b1) woven in as PE filler ----
            with ExitStack() as p1:
                qkvp0 = p1.enter_context(tc.tile_pool(name="qkvp0", bufs=1))
                attn0 = p1.enter_context(tc.tile_pool(name="attn0", bufs=1))
                expp0 = p1.enter_context(tc.tile_pool(name="expp0", bufs=2))
                mskp0 = p1.enter_context(tc.tile_pool(name="mskp0", bufs=1))
                lnp0 = p1.enter_context(tc.tile_pool(name="lnp0", bufs=3))
                xnp0 = p1.enter_context(tc.tile_pool(name="xnp0", bufs=1))
                smal0 = p1.enter_context(tc.tile_pool(name="smal0", bufs=2))
                psA0 = p1.enter_context(tc.tile_pool(name="psA0", bufs=2, space="PSUM"))
                psB0 = p1.enter_context(tc.tile_pool(name="psB0", bufs=2, space="PSUM"))
                psC0 = p1.enter_context(tc.tile_pool(name="psC0", bufs=1, space="PSUM"))
                for u in qkv_units(0, qkvp0, psA0, smal0):
                    u()
                ua = attn_units(0, expp0, mskp0, smal0, lnp0, xnp0, psB0, psC0, attn0)
                ub = qkv_units(1, qkvp1, psA0, smal0)
                weave(ua, ub)

            # ---- phase 2: attention(b1) woven with conv1(b0) ----
            hT0p = ctx.enter_context(tc.tile_pool(name="hT0p", bufs=1, side="right"))
            hT0 = hT0p.tile([128, FT, SP], BF16, tag="hT0")
            nc.gpsimd.memset(hT0[:, :, 0:4], 0.0)
            nc.gpsimd.memset(hT0[:, :, 4 + S : SP], 0.0)
            w1p = ctx.enter_context(tc.tile_pool(name="w1p", bufs=2, side="right"))
            psF = ctx.enter_context(
                tc.tile_pool(name="psF", bufs=2, space="PSUM", side="right")
            )
            with ExitStack() as p2:
                attn1 = p2.enter_context(tc.tile_pool(name="attn1", bufs=1))
                expp1 = p2.enter_context(tc.tile_pool(name="expp1", bufs=1))
                mskp1 = p2.enter_context(tc.tile_pool(name="mskp1", bufs=1))
                lnp1 = p2.enter_context(tc.tile_pool(name="lnp1", bufs=3))
                xnp1 = p2.enter_context(tc.tile_pool(name="xnp1", bufs=1))
                smal1 = p2.enter_context(tc.tile_pool(name="smal1", bufs=2))
                psB1 = p2.enter_context(tc.tile_pool(name="psB1", bufs=2, space="PSUM"))
                psC1 = p2.enter_context(tc.tile_pool(name="psC1", bufs=1, space="PSUM"))
                ua = attn_units(1, expp1, mskp1, smal1, lnp1, xnp1, psB1, psC1, attn1)
                ub = conv1_units(0, w1p, psF, hT0)
                weave(ua, ub)

        # ---- phase 3: conv1(b1) + w2 chunks, then conv2(b0) ----
        with ExitStack() as p3:
            hT1p = p3.enter_context(tc.tile_pool(name="hT1p", bufs=1))
            hT1 = hT1p.tile([128, FT, SP], BF16, tag="hT1")
            nc.gpsimd.memset(hT1[:, :, 0:4], 0.0)
            nc.gpsimd.memset(hT1[:, :, 4 + S : SP], 0.0)
            w2p = p3.enter_context(tc.tile_pool(name="w2p", bufs=1))
            w2 = w2p.tile([128, K, FT, D], BF16, tag="w2")
            w2_dmas = [
                (lambda k9=k9: nc.sync.dma_start(w2[:, k9], wc2_d[:, k9]))
                for k9 in range(K)
            ]
            psG = p3.enter_context(tc.tile_pool(name="psG", bufs=4, space="PSUM"))
            ln2 = p3.enter_context(tc.tile_pool(name="ln2", bufs=2))
            for u in conv1_units(1, w1p, psF, hT1, extra_dma=w2_dmas):
                u()
            conv2(0, psG, ln2, hT0, w2)
            # ---- phase 4 ----
            conv2(1, psG, ln2, hT1, w2)


def _build():
    if "nc" not in _CACHE:
        nc = bacc.Bacc()
        _CACHE["nc"] = _emit(nc)
    return _CACHE["nc"]


def _prep_shared(Wq, bq, Wk, bk, Wv, bv, Wo, bo, Wc1, bc1, Wc2, bc2, g1, beta1, g2, beta2):
    bf = ml_dtypes.bfloat16
    f32 = np.float32
    sh = {}
    # [H, D, DH] -> [H, DC, 128, DH] -> [128, H, DC, DH]
    f8 = ml_dtypes.float8_e4m3
    sh["wq"] = np.ascontiguousarray(
        Wq.reshape(H, DC, 128, DH).transpose(2, 0, 1, 3).astype(bf))
    sh["wk"] = np.ascontiguousarray(
        Wk.reshape(H, DC, 128, DH).transpose(2, 0, 1, 3).astype(bf))
    # V path carries 16x so the fp8 vv tile is well inside e4m3's range
    sh["wv"] = np.ascontiguousarray(
        (Wv * 16.0).reshape(H, DC, 128, DH).transpose(2, 0, 1, 3).astype(bf))
    # Wo in fp8, x64 (ON carries 16x -> psum = 1024*attn)
    wo = np.zeros((128, 4, D), dtype=f8)
    bounds = ((0, 128), (128, 192), (192, 320), (320, 384))
    for c, (r0, r1) in enumerate(bounds):
        wo[: r1 - r0, c] = np.clip(Wo[r0:r1] * 64.0, -240, 240).astype(f8)
    sh["wo"] = wo
    # [K, D, F] -> [FT, 128p(of D-chunk), K, DC, 128f]
    wc1 = Wc1.reshape(K, DC, 128, FT, 128)  # k, dc, p, ft, f
    sh["wc1"] = np.ascontiguousarray(wc1.transpose(3, 2, 0, 1, 4).astype(bf))
    # [K, F, D] -> [128p(of F-chunk), K, FT, D]
    wc2 = Wc2.reshape(K, FT, 128, D)
    sh["wc2"] = np.ascontiguousarray(wc2.transpose(2, 0, 1, 3).astype(bf))
    bqk = np.zeros((2, H, 2, 128), dtype=f32)
    for i, bb in enumerate((bq, bk)):
        for h in range(H):
            bqk[i, h, 0, :] = bb[h, :128] * 8.0
            bqk[i, h, 1, :64] = bb[h, 128:] * 8.0
    sh["bqk"] = np.ascontiguousarray(bqk.transpose(3, 0, 1, 2))
    sh["bv"] = bv.astype(f32) * 16.0
    sh["bc1t"] = np.ascontiguousarray(bc1.reshape(FT, 128).T.astype(f32))
    sh["lnc"] = np.ascontiguousarray(
        np.stack([g1, beta1, g2, beta2, bc2]).astype(f32))
    return sh


def run_sharded(inputs, trace=False):
    nc = _build()
    x = np.asarray(inputs["x"], dtype=np.float32)
    mask = np.asarray(inputs["mask"])
    sh = _prep_shared(
        *[np.asarray(inputs[k]) for k in (
            "Wq", "bq", "Wk", "bk", "Wv", "bv", "Wo", "bo",
            "Wc1", "bc1", "Wc2", "bc2", "g1", "beta1", "g2", "beta2",
        )]
    )
    bf = ml_dtypes.bfloat16
    bo = np.asarray(inputs["bo"], dtype=np.float32)
    in_maps = []
    for c in range(NCORES):
        xb = x[c * NB : (c + 1) * NB]  # [NB, S, D]
        m = {}
        # xT: [NB, 128p(of D-chunk), DC, S]
        m["xT"] = np.ascontiguousarray(
            xb.transpose(0, 2, 1).reshape(NB, DC, 128, S).transpose(0, 2, 1, 3)
        ).astype(bf)
        # xn: residual with bo folded in, x1024 to match the fp8 out-proj
        # psum scale (layernorm is scale-invariant); [NB, 128p, ST, D]
        m["xn"] = np.ascontiguousarray(
            ((xb + bo) * 1024.0).reshape(NB, ST, 128, D).transpose(0, 2, 1, 3)
        )
        mb = mask[c * NB : (c + 1) * NB]
        # mT: [NB, 128p(of k tile), ST, S_q], fp8 (0/1 exact)
        m["mT"] = np.ascontiguousarray(
            (~mb.transpose(0, 2, 1))
            .reshape(NB, ST, 128, S)
            .transpose(0, 2, 1, 3)
            .astype(ml_dtypes.float8_e4m3)
        )
        m.update(sh)
        in_maps.append(m)
    res = run_bass_kernel_spmd(nc, in_maps, core_ids=list(range(NCORES)), trace=trace)
    out = np.empty((B, S, D), dtype=np.float32)
    for c in range(NCORES):
        out[c * NB : (c + 1) * NB] = res.results[c]["y"].reshape(NB, S, D)
    return out, res


def kernel(**inputs):
    out, _ = run_sharded(inputs, trace=False)
    return out
